# revision 1
# baseline (speedup 1.0000x reference)
"""AdaptiveGraphConv Trainium2 kernel (8 NeuronCores, SPMD).

Key identity: adj = kron(ta, I_F) + kron(I_T, fa) + I_n never needs to be
materialized.  For a batch row u (reshaped U=[T,F]):
    u @ adj          = ta^T @ U + U @ fa + U
    adj @ v (col, V) = ta @ V + V @ fa^T + V
    row-sums d[t,f]  = rs_ta[t] + rs_fa[f] + 1
    x @ L            = x - S * ((S*x) @A-form),  S = (d+1e-10)^-1/2
    out_h = X @ (w0-w1) + (2/sn) * (X - S*W_pre) @ w1,
            W_pre = ta^T@Y + Y@fa + Y,  Y = S*X
sn from power iteration: u = L^3 v0, w = L u, sn = max(|u.w|/(u.u), 1).
leaky_relu mean over batch: leaky(z) = 0.6z + 0.4|z|; the 0.6z part is
rank-1 (sum_b q + sum_b k), the 0.4|z| part is one PSUM-formed tensor
abs-reduced over b (tensor_reduce(apply_absolute_value)).

fa (feature attention) depends only on weights -> host precompute.
Sharding: core c -> head c//2, batch half c%2 (x passed batch-permuted so
each core's local half is batches 0..15 of its own input; ta's batch-mean
is order independent up to f32 rounding).  Both x layouts ([t,(b f)] and
per-4-batch [(b4 f), t]) are prepared on host so no on-device transposes
of x are needed.
"""

import sys

sys.path.insert(0, "/opt/trn_rl_repo")

import numpy as np

import concourse.bass as bass
import concourse.mybir as mybir
import concourse.tile as tile
from concourse.bass_utils import run_bass_kernel_spmd

F32 = mybir.dt.float32
F32R = mybir.dt.float32r
AX = mybir.AxisListType
OP = mybir.AluOpType
AF = mybir.ActivationFunctionType

B, T, F, H, HD = 32, 128, 32, 4, 16
BL = 16  # local batch half per core
ALPHA, THRESH = 0.2, 0.01

# consts pack column offsets
OFF_ID = 0        # identity [128,128] (ident32 = [:32,:32])
OFF_BD4WQK = 128  # [kron(I4,wq) | kron(I4,wk)] [128,8]
OFF_BD4W01 = 136  # kron(I4, w0-w1) [128,64]
OFF_BD4W1N = 200  # kron(I4, -w1) [128,64]
OFF_FAT = 264     # fa.T in rows 0:32 [32,32]
OFF_V0 = 296      # v0 reshaped [128,32]
OFF_V0T = 328     # v0.T in rows 0:32 [32,128]
OFF_RSFA1 = 456   # rs_fa + 1 + 1e-10 tiled rows [128,32]
OFF_ONES = 488    # all ones [128,128]
OFF_REP4 = 616    # tile(I32, (1,4)) in rows 0:32 [32,128]
OFF_DIAGM = 744   # 1 - eye(128) [128,128]
WC = 872

TRACE = False
_NC = None
_RESULTS = None  # last BassKernelResults, for test harness


def _legalize_waits(nc, maxw=1):
    """This walrus build rejects >1 sync-wait per instruction; split the
    surplus onto same-engine NoOps placed immediately before."""
    n = 0
    for f in nc.m.functions:
        for blk in f.blocks:
            new = []
            for inst in blk.instructions:
                si = getattr(inst, "sync_info", None)
                if si is not None and len(si.on_wait) > maxw:
                    waits = list(si.on_wait)
                    for w in waits[:-maxw]:
                        nop = mybir.InstNoOp(
                            name=nc.get_next_instruction_name(),
                            engine=inst.engine, bass_nofuse=True,
                            sync_info=mybir.SyncInfo(on_wait=[w], on_update=[]))
                        new.append(nop)
                        n += 1
                    inst.sync_info = mybir.SyncInfo(
                        on_wait=waits[-maxw:], on_update=list(si.on_update))
                new.append(inst)
            try:
                blk.instructions = new
            except Exception:
                blk.instructions.clear()
                blk.instructions.extend(new)
    return n


def build_nc(stop=99):
    nc = bass.Bass()
    xa_p = nc.declare_dram_parameter("xa", [T, BL * F], F32, isOutput=False)
    xb_p = nc.declare_dram_parameter("xb4", [128, 1032], F32,
                                     isOutput=False)  # chunks + wqk
    c_p = nc.declare_dram_parameter("consts", [128, WC], F32, isOutput=False)
    cr_p = nc.declare_dram_parameter("constsr", [128, 896], F32R, isOutput=False)
    o_p = nc.declare_dram_parameter("out", [T, BL * HD], F32, isOutput=True)

    from contextlib import ExitStack

    with tile.TileContext(nc) as tc, ExitStack() as ctx:
        sb1 = ctx.enter_context(tc.tile_pool(name="sb1", bufs=1))
        sbw = ctx.enter_context(tc.tile_pool(name="sbw", bufs=3))
        psA = ctx.enter_context(tc.tile_pool(name="psA", bufs=2, space="PSUM"))
        psB = ctx.enter_context(tc.tile_pool(name="psB", bufs=4, space="PSUM"))

        # ---------- input loads ----------
        XB4 = sb1.tile([128, 1032], F32, tag="xb4")  # chunks + wqk
        nc.sync.dma_start(out=XB4[:, 0:516], in_=xb_p[:, 0:516])
        nc.scalar.dma_start(out=XB4[:, 516:1032], in_=xb_p[:, 516:1032])
        constsr = sb1.tile([128, 896], F32R, tag="constsr")
        consts = sb1.tile([128, WC], F32, tag="consts")
        ind512r = constsr[0:32, 0:512]
        identr = constsr[:, 640:768]
        bd4far = constsr[:, 768:896]

        ident = consts[:, OFF_ID:OFF_ID + 128]
        ident32 = consts[0:32, OFF_ID:OFF_ID + 32]
        bd4wqk = XB4[:, 1024:1032]
        bd4w01 = consts[:, OFF_BD4W01:OFF_BD4W01 + 64]
        bd4w1n = consts[:, OFF_BD4W1N:OFF_BD4W1N + 64]
        diagm = consts[:, OFF_DIAGM:OFF_DIAGM + 128]
        faT = consts[0:32, OFF_FAT:OFF_FAT + 32]
        v0 = consts[:, OFF_V0:OFF_V0 + 32]
        v0T = consts[0:32, OFF_V0T:OFF_V0T + 128]
        rsfa1 = consts[:, OFF_RSFA1:OFF_RSFA1 + 32]
        ones_row = consts[0:1, OFF_ONES:OFF_ONES + 128]
        ones_col = consts[:, OFF_ONES:OFF_ONES + 1]
        rep4 = consts[0:32, OFF_REP4:OFF_REP4 + 128]

        # preload the natural_log_exp table set (has Ln/Exp/Abs/Copy) off
        # the critical path so later activations don't swap tables
        dummy0 = sbw.tile([1, 1], F32, tag="dummy0")
        nc.vector.memset(dummy0, 1.0)
        dummy = sbw.tile([1, 1], F32, tag="dummy")
        nc.scalar.activation(dummy, dummy0, AF.Ln)



        XA = sb1.tile([128, BL * F], F32, tag="xa")  # [t, (b f)] local half


        if stop <= 1:
            return nc
        # ---------- q/kk: qkk[t, 0:32]=q (b asc), [t, 32:64]=kk ----------
        # qkk cols per chunk c: [q(b4) x4 | kk(b4) x4] at 8c..8c+8
        qkkP = psB.tile([128, 64], F32, tag="ps")
        for c in range(8):
            xbc = XB4[:, 128 * c:128 * (c + 1)]
            nc.tensor.matmul(qkkP[:, 8 * c:8 * c + 8], xbc, bd4wqk,
                             start=True, stop=True)
        qkk = sb1.tile([128, 64], F32R, tag="qkk")
        nc.vector.tensor_copy(qkk, qkkP)
        nc.gpsimd.dma_start(out=constsr, in_=cr_p[:, :])
        nc.gpsimd.dma_start(out=consts, in_=c_p[:, :])
        q_ap = bass.AP(tensor=qkk.tensor, offset=qkk[:, 0:1].offset,
                       ap=[qkk.ap[0], [8, 8], [1, 4]])
        kk_ap = bass.AP(tensor=qkk.tensor, offset=qkk[:, 4:5].offset,
                        ap=[qkk.ap[0], [8, 8], [1, 4]])

        # kk flatten: 2 tiles x rows {0,32}, 1024 cols each (blocks of 512)
        kkA = sb1.tile([33, 1024], F32R, tag="kkA")
        kkB = sb1.tile([33, 1024], F32R, tag="kkB")
        nc.sync.dma_start(
            out=bass.AP(tensor=kkA.tensor, offset=kkA.offset,
                        ap=[[32768, 2], [1, 1024]]),
            in_=bass.AP(tensor=qkk.tensor, offset=qkk[0:64, 4:5].offset,
                        ap=[[qkk.ap[0][0], 64], [8, 8], [1, 4]]))
        nc.scalar.dma_start(
            out=bass.AP(tensor=kkB.tensor, offset=kkB.offset,
                        ap=[[32768, 2], [1, 1024]]),
            in_=bass.AP(tensor=qkk.tensor, offset=qkk[64:128, 4:5].offset,
                        ap=[[qkk.ap[0][0], 64], [8, 8], [1, 4]]))
        qsel = sbw.tile([128, 32], F32R, tag="qsel")
        nc.vector.tensor_copy(qsel, q_ap)
        qT = sb1.tile([32, 128], F32R, tag="qT")
        ptq = psB.tile([32, 128], F32R, tag="ps")
        nc.tensor.transpose(ptq, qsel, identr)
        nc.vector.tensor_copy(qT, ptq)

        # linear part of leaky decomposition
        Sq = sb1.tile([128, 1], F32, tag="Sq")
        nc.vector.reduce_sum(out=Sq, in_=q_ap, axis=AX.XY)
        Sk_col = sb1.tile([128, 1], F32, tag="Skc")
        nc.vector.reduce_sum(out=Sk_col, in_=kk_ap, axis=AX.XY)
        ptk = psB.tile([1, 128], F32, tag="ps")
        nc.tensor.transpose(ptk, Sk_col, ident)
        Skrow = sb1.tile([1, 128], F32, tag="skrow")
        nc.scalar.copy(Skrow, ptk)
        skb = psB.tile([128, 128], F32, tag="ps")
        nc.tensor.matmul(skb, ones_row, Skrow, start=True, stop=True)
        tl_lin = sb1.tile([128, 128], F32, tag="tllin")
        nc.vector.tensor_scalar(tl_lin, skb, Sq, 0.6 / 32.0,
                                op0=OP.add, op1=OP.mult)

        if stop <= 2:
            return nc
        for _ in range(3):
            pj = psB.tile([128, 512], F32, tag="ps")
            nc.tensor.matmul(pj, identr, constsr[:, 0:512], start=True,
                             stop=True)
        # ---------- E = q[b,t1]+kk[b,t2] in PSUM, abs-reduce over b -------
        A_abs = sb1.tile([128, 128], F32, tag="aabs")
        for k in range(4):
            eps_ = psA.tile([128, 1024], F32, tag="ps_e")
            for hf in range(2):
                i = 2 * k + hf
                tl_ = kkA if i < 4 else kkB
                ii = i % 4
                beta = 32 * (ii // 2)
                sl = eps_[:, 512 * hf:512 * hf + 512]
                ones_b = constsr[beta:beta + 1, 512:640]
                nc.tensor.matmul(
                    sl, ones_b,
                    tl_[beta:beta + 1, 512 * (ii % 2):512 * (ii % 2) + 512],
                    start=True, stop=False)  # noqa
                nc.tensor.matmul(sl, qT, ind512r, start=False, stop=True)
            nc.vector.tensor_reduce(
                out=A_abs[:, 32 * k:32 * k + 32],
                in_=eps_.rearrange("p (t b) -> p t b", b=32),
                axis=AX.X, op=OP.add, apply_absolute_value=True)

        tl = sb1.tile([128, 128], F32, tag="tl")
        nc.vector.scalar_tensor_tensor(tl, A_abs, 0.4 / 32.0, tl_lin,
                                       op0=OP.mult, op1=OP.add)

        if stop <= 3:
            return nc
        # ---------- softmax + threshold + zero diag -> ta ----------
        negmax = sb1.tile([128, 1], F32, tag="negmax")
        nc.vector.tensor_reduce(out=negmax, in_=tl, axis=AX.X, op=OP.max,
                                negate=True)
        e_sb = sbw.tile([128, 128], F32, tag="esb")
        ssum = sbw.tile([128, 1], F32, tag="ssum")
        nc.scalar.activation(e_sb, tl, AF.Exp, bias=negmax, scale=1.0,
                             accum_out=ssum)
        rsum = sbw.tile([128, 1], F32, tag="rsum")
        nc.vector.reciprocal(rsum, ssum)
        ta1 = sbw.tile([128, 128], F32, tag="ta1")
        nc.vector.scalar_tensor_tensor(ta1, e_sb, rsum, diagm,
                                       op0=OP.mult, op1=OP.mult)
        ta = sb1.tile([128, 128], F32, tag="ta")
        rsta = sb1.tile([128, 1], F32, tag="rsta")
        nc.vector.scalar_tensor_tensor(ta, ta1, THRESH, ta1,
                                       op0=OP.is_gt, op1=OP.mult,
                                       accum_out=rsta)

        # ---------- S = (rs_ta + rs_fa + 1 + 1e-10)^-1/2 = exp(-.5 ln d) --
        lnd = sbw.tile([128, 32], F32, tag="lnd")
        nc.scalar.activation(lnd, rsfa1, AF.Ln, bias=rsta)
        S = sb1.tile([128, 32], F32, tag="S")
        nc.scalar.activation(S, lnd, AF.Exp, scale=-0.5)
        dd = sb1.tile([128, 32], F32, tag="dd")  # = d + 1e-10 (for u.w)
        nc.vector.tensor_scalar_add(dd, rsfa1, rsta)

        # ST [f, t], SB4 [(b4 f), t]
        ptS = psB.tile([32, 128], F32, tag="ps")
        nc.tensor.transpose(ptS, S, ident)
        ST = sb1.tile([32, 128], F32, tag="st")
        nc.vector.tensor_copy(ST, ptS)
        psb4 = psB.tile([128, 128], F32, tag="ps")
        nc.tensor.matmul(psb4, rep4, ST, start=True, stop=True)
        SB4 = sb1.tile([128, 128], F32, tag="sb4")
        nc.scalar.copy(SB4, psb4)
        # taT for column-form applies
        ptt = psB.tile([128, 128], F32, tag="ps")
        nc.tensor.transpose(ptt, ta, ident)
        taT = sb1.tile([128, 128], F32, tag="taT")
        nc.vector.tensor_copy(taT, ptt)

        if stop <= 4:
            return nc
        # ---------- heavy path prep (independent of sn) ----------
        nc.scalar.dma_start(out=XA, in_=xa_p[:, :])
        S_bc = bass.AP(tensor=S.tensor, offset=S.offset,
                       ap=[S.ap[0], [0, BL], S.ap[1]])
        YA = sbw.tile([128, BL * F], F32, tag="ya")
        nc.vector.tensor_tensor(
            YA.rearrange("p (b f) -> p b f", f=F),
            XA.rearrange("p (b f) -> p b f", f=F),
            S_bc, OP.mult)
        sb4_bc = bass.AP(tensor=SB4.tensor, offset=SB4.offset,
                         ap=[SB4.ap[0], [0, 4], SB4.ap[1]])
        yb4 = sbw.tile([128, 512], F32R, tag="yb4")
        nc.vector.tensor_tensor(
            yb4.rearrange("p (c t) -> p c t", t=128),
            XB4[:, 0:512].rearrange("p (c t) -> p c t", t=128),
            sb4_bc, OP.mult)
        wc = psA.tile([128, 512], F32, tag="ps_e")
        nc.tensor.matmul(wc, bd4far, yb4, start=True, stop=False)
        nc.tensor.matmul(wc, identr, yb4, start=False, stop=True)
        for j in range(4):
            # (ta^T Y)^T for chunk j, directly in B4 layout
            nc.tensor.matmul(wc[:, 128 * j:128 * (j + 1)],
                             YA[:, 128 * j:128 * (j + 1)], ta,
                             start=False, stop=True,
                             skip_group_check=True)
        tmp = sbw.tile([128, 512], F32, tag="tmp")
        nc.vector.tensor_tensor(
            tmp.rearrange("p (c t) -> p c t", t=128),
            wc.rearrange("p (c t) -> p c t", t=128),
            sb4_bc, OP.mult)  # S*W_pre (B4)
        n1 = sb1.tile([128, 512], F32, tag="n1")
        nc.gpsimd.tensor_tensor(n1, tmp, XB4[:, 0:512], OP.subtract)
        TMP = n1

        if stop <= 5:
            return nc
        # ---------- power iteration on z = S*v (no in-loop scaling) -----
        S2 = sb1.tile([128, 32], F32, tag="S2")
        nc.vector.tensor_tensor(S2, S, S, OP.mult)
        S2T = sb1.tile([32, 128], F32, tag="S2T")
        nc.gpsimd.tensor_tensor(S2T, ST, ST, OP.mult)
        ztf0 = sb1.tile([128, 32], F32, tag="ztf0")
        nc.gpsimd.tensor_tensor(ztf0, v0, S, OP.mult)
        zft0 = sb1.tile([32, 128], F32, tag="zft0")
        nc.gpsimd.tensor_tensor(zft0, v0T, ST, OP.mult)
        Ztf, Zft = ztf0, zft0
        u_tf = None
        for it in range(4):
            pva = psB.tile([128, 32], F32, tag="ps")
            nc.tensor.matmul(pva, taT, Ztf, start=True, stop=False)
            nc.tensor.matmul(pva, Zft, faT, start=False, stop=False)
            nc.tensor.matmul(pva, ident, Ztf, start=False, stop=True)
            m1 = sbw.tile([128, 32], F32, tag="m1")
            nc.vector.tensor_tensor(m1, pva, S2, OP.mult)
            zn = sb1.tile([128, 32], F32, tag=f"z_{it + 1}")
            nc.vector.tensor_tensor(zn, Ztf, m1, OP.subtract)
            if it < 3:
                pvb = psB.tile([32, 128], F32, tag="ps")
                nc.tensor.matmul(pvb, Ztf, taT, start=True, stop=False)
                nc.tensor.matmul(pvb, faT, Zft, start=False, stop=False)
                nc.tensor.matmul(pvb, ident32, Zft, start=False, stop=True)
                m2 = sbw.tile([32, 128], F32, tag="m2")
                nc.vector.tensor_tensor(m2, pvb, S2T, OP.mult)
                znT = sb1.tile([32, 128], F32, tag=f"zt_{it + 1}")
                nc.vector.tensor_tensor(znT, Zft, m2, OP.subtract)
                Zft = znT
            Ztf = zn
            if it == 2:
                u_tf = zn
        w_tf = Ztf

        # u.w and u.u with 1/S^2 = dd weighting (z = S*v)
        uw = sbw.tile([128, 32], F32, tag="uw")
        nc.vector.tensor_tensor(uw, u_tf, w_tf, OP.mult)
        uwd = sbw.tile([128, 32], F32, tag="uwd")
        nc.vector.tensor_tensor(uwd, uw, dd, OP.mult)
        uu = sbw.tile([128, 32], F32, tag="uu")
        nc.vector.tensor_tensor(uu, u_tf, u_tf, OP.mult)
        uud = sbw.tile([128, 32], F32, tag="uud")
        nc.vector.tensor_tensor(uud, uu, dd, OP.mult)
        duo = sbw.tile([128, 2], F32, tag="duo")
        nc.vector.reduce_sum(out=duo[:, 0:1], in_=uwd, axis=AX.X)
        nc.vector.reduce_sum(out=duo[:, 1:2], in_=uud, axis=AX.X)
        ps0 = psB.tile([1, 1], F32, tag="ps")
        nc.tensor.matmul(ps0, duo[:, 0:1], ones_col, start=True, stop=True)
        ps1 = psB.tile([1, 1], F32, tag="ps")
        nc.tensor.matmul(ps1, duo[:, 1:2], ones_col, start=True, stop=True)
        a0 = sbw.tile([1, 1], F32, tag="a0")
        nc.vector.tensor_reduce(out=a0, in_=ps0, axis=AX.X, op=OP.max,
                                apply_absolute_value=True)
        rb = sbw.tile([1, 1], F32, tag="rb")
        nc.vector.reciprocal(rb, ps1)
        sn0 = sbw.tile([1, 1], F32, tag="sn0")
        nc.vector.tensor_tensor(sn0, a0, rb, OP.mult)  # |uw|/uu = sn
        rsn = sbw.tile([1, 1], F32, tag="rsn")
        nc.vector.reciprocal(rsn, sn0)
        ts2 = sbw.tile([1, 1], F32, tag="ts2")
        # 2/max(sn,1) = min(2/sn, 2)
        nc.vector.tensor_scalar(ts2, rsn, 2.0, 2.0, op0=OP.mult, op1=OP.min)
        pst = psB.tile([128, 1], F32, tag="ps")
        nc.tensor.matmul(pst, ones_row, ts2, start=True, stop=True)
        ts_col = sb1.tile([128, 1], F32, tag="tscol")
        nc.vector.tensor_copy(ts_col, pst)

        if stop <= 6:
            return nc
        # ---------- final combine + output matmuls ----------
        out_sb = sb1.tile([128, BL * HD], F32, tag="outsb")
        gb4n = sbw.tile([128, 512], F32, tag="gb4n")
        nc.vector.tensor_scalar_mul(gb4n, TMP, ts_col)  # -G = ts*(S*Wp-X)
        for j in range(4):
            ops_ = psB.tile([128, 64], F32, tag="ps")
            nc.tensor.matmul(ops_, XB4[:, 128 * j:128 * (j + 1)], bd4w01,
                             start=True, stop=False)
            nc.tensor.matmul(ops_, gb4n[:, 128 * j:128 * (j + 1)], bd4w1n,
                             start=False, stop=True)
            if j % 2 == 0:
                nc.scalar.copy(out_sb[:, 64 * j:64 * j + 64], ops_)
            else:
                nc.vector.tensor_copy(out_sb[:, 64 * j:64 * j + 64], ops_)
        nc.sync.dma_start(out=o_p[:, 0:128], in_=out_sb[:, 0:128])
        nc.scalar.dma_start(out=o_p[:, 128:256], in_=out_sb[:, 128:256])

    return nc


def _host_consts(weight, temporal_query, temporal_key, feature_factor):
    """Per-head consts pack [128, WC] f32 + shared IND [32, 4096]."""
    import jax

    import jax.numpy as jnp

    eye4 = np.eye(4, dtype=np.float32)
    packs = []
    crs = []
    for h in range(H):
        U = feature_factor[h, 0]
        V = feature_factor[h, 1]
        fa0 = jax.nn.softmax(jax.nn.leaky_relu(jnp.asarray(U) @ jnp.asarray(V).T,
                                               ALPHA), axis=1)
        fa = np.asarray(jnp.where((fa0 > THRESH) & ~jnp.eye(F, dtype=bool),
                                  fa0, 0.0), dtype=np.float32)
        w0 = weight[0, h].astype(np.float32)
        w1 = weight[1, h].astype(np.float32)
        wq = (w0 @ temporal_query[h]).astype(np.float32)
        wk = (w0 @ temporal_key[h]).astype(np.float32)
        v0 = np.asarray(jax.random.normal(
            jax.random.fold_in(jax.random.key(42), h), (T * F, 1),
            jnp.float32)).reshape(T, F)

        c = np.zeros((128, WC), dtype=np.float32)
        c[:, OFF_ID:OFF_ID + 128] = np.eye(128, dtype=np.float32)
        c[:, OFF_BD4WQK:OFF_BD4WQK + 4] = np.kron(eye4, wq[:, None])
        c[:, OFF_BD4WQK + 4:OFF_BD4WQK + 8] = np.kron(eye4, wk[:, None])
        c[:, OFF_BD4W01:OFF_BD4W01 + 64] = np.kron(eye4, w0 - w1)
        c[:, OFF_BD4W1N:OFF_BD4W1N + 64] = np.kron(eye4, -w1)
        c[0:32, OFF_FAT:OFF_FAT + 32] = fa.T
        c[:, OFF_V0:OFF_V0 + 32] = v0
        c[0:32, OFF_V0T:OFF_V0T + 128] = v0.T
        c[:, OFF_RSFA1:OFF_RSFA1 + 32] = np.tile(
            (fa.sum(axis=1) + 1.0 + 1e-10)[None, :].astype(np.float32),
            (128, 1))
        c[:, OFF_ONES:OFF_ONES + 128] = 1.0
        c[0:32, OFF_REP4:OFF_REP4 + 128] = np.tile(
            np.eye(32, dtype=np.float32), (1, 4))
        c[:, OFF_DIAGM:OFF_DIAGM + 128] = 1.0 - np.eye(128, dtype=np.float32)
        packs.append(c)
        crs.append(np.kron(eye4, fa))

    crl = []
    for h in range(H):
        cr = np.zeros((128, 896), dtype=np.float32)
        cr[0:32, 0:512] = np.tile(np.eye(32, dtype=np.float32), (1, 16))
        cr[:, 512:640] = 1.0
        cr[:, 640:768] = np.eye(128, dtype=np.float32)
        cr[:, 768:896] = crs[h]
        crl.append(cr)
    return packs, crl


def kernel(x, weight, bias, temporal_query, temporal_key, feature_factor):
    global _NC, _RESULTS
    x = np.ascontiguousarray(np.asarray(x, dtype=np.float32))
    weight = np.asarray(weight, dtype=np.float32)
    bias = np.asarray(bias, dtype=np.float32)
    temporal_query = np.asarray(temporal_query, dtype=np.float32)
    temporal_key = np.asarray(temporal_key, dtype=np.float32)
    feature_factor = np.asarray(feature_factor, dtype=np.float32)

    packs, crl = _host_consts(weight, temporal_query, temporal_key,
                              feature_factor)
    if _NC is None:
        _NC = build_nc()
        _legalize_waits(_NC)

    in_maps = []
    for c in range(8):
        h, half = c // 2, c % 2
        if half == 0:
            xp_ = x
        else:
            xp_ = np.concatenate([x[16:32], x[0:16]], axis=0)
        xa = np.ascontiguousarray(
            xp_[0:BL].transpose(1, 0, 2).reshape(T, BL * F))
        xb4 = np.ascontiguousarray(np.concatenate(
            [xp_[4 * cc:4 * cc + 4].transpose(0, 2, 1).reshape(128, 128)
             for cc in range(8)] + [packs[h][:, OFF_BD4WQK:OFF_BD4WQK + 8]],
            axis=1))
        in_maps.append({"xa": xa, "xb4": xb4, "consts": packs[h],
                        "constsr": crl[h]})

    _RESULTS = run_bass_kernel_spmd(_NC, in_maps, core_ids=list(range(8)),
                                    trace=TRACE)
    res = _RESULTS.results

    out = np.zeros((B, T, H * HD), dtype=np.float32)
    for c in range(8):
        h, half = c // 2, c % 2
        oc = np.asarray(res[c]["out"]).reshape(T, BL, HD)
        out[16 * half:16 * half + 16, :, HD * h:HD * h + HD] = \
            oc.transpose(1, 0, 2)
    out = out + bias.reshape(1, 1, H * HD)
    return out

